# revision 24
# baseline (speedup 1.0000x reference)
"""Top-k masking sparse projection on 8 TRN2 NeuronCores (Bass/Tile).

out = x * (x >= kth_largest_per_row(x)),  x = input @ weight.T
Data-parallel over the batch dim: each core handles 512 of 4096 rows.

Math: weight is binary (0/1), so splitting the input into bf16 hi+lo parts
makes both bf16 matmuls exact products; fp32 PSUM accumulation gives x to
~1e-6 abs, far below the typical spacing (~6e-3) between the 32nd/33rd
order statistics, so the kept set matches the fp32 reference.

Top-k per row (10240 wide, rows on partitions): 32 x max8 over contiguous
segments of 320 -> 256 candidates (a segment holding >8 of the row's top-32
has probability ~1e-6 per row); then 4 rounds of max8 + match_replace on the
candidates yield the exact 32nd-largest value; one fused
scalar_tensor_tensor pass applies (x >= t) * x in place.
"""

import numpy as np
import ml_dtypes
from contextlib import ExitStack

BATCH, IN_FEATURES, OUT_FEATURES, N_CORES = 4096, 512, 10240, 8
ROWS = BATCH // N_CORES          # rows per core
P = 128                          # partitions
RB = ROWS // P                   # row blocks per core
NOC = OUT_FEATURES // 512        # output chunks of 512
KT = IN_FEATURES // P            # contraction tiles
NSEG = 32                        # top-k candidate segments per row
SEG = OUT_FEATURES // NSEG       # segment length (320)

_graph_cache = {}
_w_cache = {}


def _build(k, n_iter=1):
    """Build the SPMD Bass graph for top-k threshold k (same on all cores).

    n_iter > 1 unrolls the whole computation (including input/weight DMA)
    back-to-back in one NEFF, for slope-based hardware timing.
    """
    import concourse.bass as bass
    import concourse.bacc as bacc
    import concourse.mybir as mybir
    from concourse import tile

    f32 = mybir.dt.float32
    bf16 = mybir.dt.bfloat16
    nrounds = (k + 7) // 8
    assert 1 <= k <= 64, f"unsupported hash_length {k}"

    nc = bacc.Bacc()
    # act: packed transposed activations, col = split*KT*ROWS + kt*ROWS + r
    act_d = nc.declare_dram_parameter("act", [P, 2 * KT * ROWS], bf16, isOutput=False)
    wt_d = nc.declare_dram_parameter("wt", [IN_FEATURES, OUT_FEATURES], bf16, isOutput=False)
    out_d = nc.declare_dram_parameter("out", [ROWS, OUT_FEATURES], bf16, isOutput=True)

    with tile.TileContext(nc) as tc, ExitStack() as ctx:
        wpool = ctx.enter_context(tc.tile_pool(name="w", bufs=1))
        apool = ctx.enter_context(tc.tile_pool(name="a", bufs=1))
        xpool = ctx.enter_context(tc.tile_pool(name="x", bufs=2))
        ypool = ctx.enter_context(tc.tile_pool(name="y", bufs=1))
        pspool = ctx.enter_context(tc.tile_pool(name="ps", bufs=8, space="PSUM"))
        spool = ctx.enter_context(tc.tile_pool(name="small", bufs=2))

        QW = OUT_FEATURES // 4
        OC_PER_Q = NOC // 4
        wt_src = wt_d[:, :].rearrange("(kt p) o -> p kt o", p=P)

        def one_iter():
            # all activations in one DMA (one semaphore for every matmul lhsT)
            a_t = apool.tile([P, 2 * KT * ROWS], bf16, tag="a", name="a_t")
            nc.sync.dma_start(out=a_t[:], in_=act_d[:, :])

            # weights: one [128, KT*OUT_FEATURES] tile, kt-major columns;
            # 4 DMAs (one per outf quarter), each covering all KT k-tiles
            w_t = wpool.tile([P, KT * OUT_FEATURES], bf16, tag="w", name="w_t")
            wt_dst = w_t[:].rearrange("p (kt o) -> p kt o", kt=KT)
            for q in range(4):
                nc.sync.dma_start(
                    out=wt_dst[:, :, q * QW:(q + 1) * QW],
                    in_=wt_src[:, :, q * QW:(q + 1) * QW],
                )

            def lhs_ap(split, kt, rb):
                c = (split * KT + kt) * ROWS + rb * P
                return a_t[:, c:c + P]

            # The Matmult ISA struct has a single sync-wait slot, so matmuls
            # must never need both a DMA wait and a PSUM-WAR wait. Gate each
            # input DMA with a throwaway ldweights carrying the DMA wait.
            nc.tensor.ldweights(weights=a_t[:, 0:P])

            G_OC = 4  # output chunks per stationary-reuse group (PSUM banks)
            for rb in range(RB):
                xt = xpool.tile([P, OUT_FEATURES], f32, tag="xt", name="xt")
                rsl = slice(rb * P, (rb + 1) * P)
                for og in range(NOC // G_OC):
                    ocs = range(og * G_OC, (og + 1) * G_OC)
                    if rb == 0:
                        # gate the weight-quarter DMAs this group first touches
                        for q in range(4):
                            if any(oc // OC_PER_Q == q for oc in ocs) and \
                               any(oc % OC_PER_Q == 0 for oc in ocs if oc // OC_PER_Q == q):
                                nc.tensor.ldweights(weights=w_t[:, q * QW:q * QW + P])
                    pts = [pspool.tile([P, 512], f32, tag="pt", name="pt")
                           for _ in ocs]
                    n = 0
                    for kt in range(KT):
                        for split in (0, 1):
                            for j, oc in enumerate(ocs):
                                nc.tensor.matmul(
                                    pts[j][:],
                                    lhsT=lhs_ap(split, kt, rb),
                                    rhs=w_t[:, kt * OUT_FEATURES + oc * 512:
                                            kt * OUT_FEATURES + (oc + 1) * 512],
                                    start=(n == 0),
                                    stop=(n == 2 * KT - 1),
                                )
                            n += 1
                    for j, oc in enumerate(ocs):
                        nc.scalar.copy(xt[:, oc * 512:(oc + 1) * 512], pts[j][:])

                # segmented top-8 -> 256 candidates per row
                cand = spool.tile([P, NSEG * 8], f32, tag="cand", name="cand")
                for s in range(NSEG):
                    nc.vector.max(cand[:, 8 * s:8 * (s + 1)],
                                  xt[:, SEG * s:SEG * (s + 1)])
                # peel 8 at a time to reach the k-th largest value
                t8 = spool.tile([P, 8 * nrounds], f32, tag="t8", name="t8")
                for r in range(nrounds):
                    nc.vector.max(t8[:, 8 * r:8 * (r + 1)], cand[:])
                    if r < nrounds - 1:
                        nc.vector.match_replace(
                            cand[:], t8[:, 8 * r:8 * (r + 1)], cand[:], -1e30
                        )
                ti = 8 * (nrounds - 1) + (k - 1) % 8
                thresh = t8[:, ti:ti + 1]
                # y = (x >= t) * x in one DVE pass; separate bf16 tile so the
                # out-DMA has exactly one wait (single-wait-slot DMA struct)
                yt = ypool.tile([P, OUT_FEATURES], bf16, tag="yt", name="yt")
                nc.vector.scalar_tensor_tensor(
                    out=yt[:], in0=xt[:], scalar=thresh, in1=xt[:],
                    op0=mybir.AluOpType.is_ge, op1=mybir.AluOpType.mult,
                )
                nc.gpsimd.dma_start(out=out_d[rsl, :], in_=yt[:])

        for _ in range(n_iter):
            one_iter()

    nc.compile()
    return nc


def _get_graph(k, n_iter=1):
    key = (k, n_iter)
    if key not in _graph_cache:
        _graph_cache[key] = _build(k, n_iter)
    return _graph_cache[key]


def _prep_weight(weight):
    key = (id(weight), weight.shape)
    if key not in _w_cache:
        _w_cache.clear()
        wt = np.ascontiguousarray(np.asarray(weight, np.float32).T)
        _w_cache[key] = wt.astype(ml_dtypes.bfloat16)
    return _w_cache[key]


def _make_in_maps(input, weight):
    inp = np.asarray(input, np.float32)
    wt = _prep_weight(weight)
    inpT = np.ascontiguousarray(inp.T)            # [IN, BATCH]
    ah = inpT.astype(ml_dtypes.bfloat16)
    al = (inpT - ah.astype(np.float32)).astype(ml_dtypes.bfloat16)
    # pack [IN, BATCH] -> per-core [P, 2*KT*ROWS], col = split*KT*ROWS + kt*ROWS + r
    def pack(a, c):
        s = a[:, c * ROWS:(c + 1) * ROWS]                      # [IN, ROWS]
        return s.reshape(KT, P, ROWS).transpose(1, 0, 2).reshape(P, KT * ROWS)
    in_maps = []
    for c in range(N_CORES):
        in_maps.append({
            "act": np.ascontiguousarray(
                np.concatenate([pack(ah, c), pack(al, c)], axis=1)),
            "wt": wt,
        })
    return in_maps


def run_spmd(input, weight, hash_length, trace=False):
    """Run the SPMD kernel; returns (full_output, BassKernelResults)."""
    from concourse.bass_utils import run_bass_kernel_spmd
    k = int(hash_length)
    nc = _get_graph(k)
    in_maps = _make_in_maps(input, weight)
    res = run_bass_kernel_spmd(nc, in_maps, core_ids=list(range(N_CORES)), trace=trace)
    out = np.concatenate(
        [res.results[c]["out"].astype(np.float32) for c in range(N_CORES)], axis=0)
    return out, res


def kernel(input, weight, hash_length):
    out, _ = run_spmd(input, weight, hash_length, trace=False)
    return out


def make_bench_fn(input, weight, hash_length, n_iter):
    """Cached jitted shard_map over the n_iter-unrolled NEFF, with inputs
    uploaded once (not donated), for repeat-dispatch wall timing."""
    import jax
    import numpy as np_
    from jax.sharding import Mesh, PartitionSpec
    from jax.experimental.shard_map import shard_map
    from concourse import bass2jax
    import concourse.mybir as mybir

    bass2jax.install_neuronx_cc_hook()
    k = int(hash_length)
    nc = _get_graph(k, n_iter)
    in_maps = _make_in_maps(input, weight)

    part_name = nc.partition_id_tensor.name if nc.partition_id_tensor else None
    in_names, out_names, out_avals, zero_outs = [], [], [], []
    for alloc in nc.m.functions[0].allocations:
        if not isinstance(alloc, mybir.MemoryLocationSet):
            continue
        name = alloc.memorylocations[0].name
        if alloc.kind == "ExternalInput":
            if name != part_name:
                in_names.append(name)
        elif alloc.kind == "ExternalOutput":
            shape = tuple(alloc.tensor_shape)
            dtype = mybir.dt.np(alloc.dtype)
            out_names.append(name)
            out_avals.append(jax.core.ShapedArray(shape, dtype))
            zero_outs.append(np_.zeros((N_CORES * shape[0], *shape[1:]), dtype))
    n_params = len(in_names)
    all_names = in_names + out_names
    if part_name is not None:
        all_names = all_names + [part_name]

    def _body(*args):
        operands = list(args)
        if part_name is not None:
            operands.append(bass2jax.partition_id_tensor())
        outs = bass2jax._bass_exec_p.bind(
            *operands,
            out_avals=tuple(out_avals),
            in_names=tuple(all_names),
            out_names=tuple(out_names),
            lowering_input_output_aliases=(),
            sim_require_finite=True,
            sim_require_nnan=True,
            nc=nc,
        )
        return tuple(outs)

    devices = jax.devices()[:N_CORES]
    mesh = Mesh(np_.asarray(devices), ("core",))
    nin = n_params + len(out_names)
    fn = jax.jit(
        shard_map(_body, mesh=mesh,
                  in_specs=(PartitionSpec("core"),) * nin,
                  out_specs=(PartitionSpec("core"),) * len(out_names),
                  check_rep=False),
        keep_unused=True,
    )
    concat_in = [
        np_.concatenate([in_maps[c][nm] for c in range(N_CORES)], axis=0)
        for nm in in_names
    ]
    dev_args = [jax.device_put(a) for a in (*concat_in, *zero_outs)]
    jax.block_until_ready(dev_args)
    return fn, dev_args


# revision 41
# speedup vs baseline: 1.8452x; 1.8452x over previous
"""Top-k masking sparse projection on 8 TRN2 NeuronCores (Bass/Tile).

out = x * (x >= kth_largest_per_row(x)),  x = input @ weight.T
Data-parallel over the batch dim: each core handles 512 of 4096 rows.

Math: weight is binary (0/1), so splitting the input into bf16 hi+lo parts
makes both bf16 matmuls exact products; fp32 PSUM accumulation gives x to
~1e-6 abs, far below the typical spacing (~6e-3) between the 32nd/33rd
order statistics, so the kept set matches the fp32 reference.

Top-k per row (10240 wide, rows on partitions): 32 x max8 over contiguous
segments of 320 -> 256 candidates (a segment holding >8 of the row's top-32
has probability ~1e-6 per row); then 4 rounds of max8 + match_replace on the
candidates yield the exact 32nd-largest value; one fused
scalar_tensor_tensor pass applies (x >= t) * x in place.
"""

import numpy as np
import ml_dtypes
from contextlib import ExitStack

BATCH, IN_FEATURES, OUT_FEATURES, N_CORES = 4096, 512, 10240, 8
ROWS = BATCH // N_CORES          # rows per core
P = 128                          # partitions
RB = ROWS // P                   # row blocks per core
NOC = OUT_FEATURES // 512        # output chunks of 512
KT = IN_FEATURES // P            # contraction tiles
NSEG = 32                        # top-k candidate segments per row
SEG = OUT_FEATURES // NSEG       # segment length (320)

_graph_cache = {}
_w_cache = {}

# tuning knobs (affect graph build; bench sweeps override these)
GROUP_OC = 5      # output chunks sharing one stationary load (1 = no grouping)
PSUM_BUFS = 8
OUT_DMA_ENGINE = "gpsimd"  # "sync" | "gpsimd"
PROBE = ""        # "" | "noscan" (skip max8 scan+rounds) | "nodve" (skip scan+stt)
Y_SPLIT = 2       # pieces the masked bf16 output is written/DMA'd in


def _build(k, n_iter=1):
    """Build the SPMD Bass graph for top-k threshold k (same on all cores).

    n_iter > 1 unrolls the whole computation (including input/weight DMA)
    back-to-back in one NEFF, for slope-based hardware timing.
    """
    import concourse.bass as bass
    import concourse.bacc as bacc
    import concourse.mybir as mybir
    from concourse import tile

    f32 = mybir.dt.float32
    bf16 = mybir.dt.bfloat16
    nrounds = (k + 7) // 8
    assert 1 <= k <= 64, f"unsupported hash_length {k}"

    nc = bacc.Bacc()
    # act: packed transposed activations, col = split*KT*ROWS + kt*ROWS + r
    act_d = nc.declare_dram_parameter("act", [P, 2 * KT * ROWS], bf16, isOutput=False)
    wt_d = nc.declare_dram_parameter("wt", [IN_FEATURES, OUT_FEATURES], bf16, isOutput=False)
    out_d = nc.declare_dram_parameter("out", [ROWS, OUT_FEATURES], bf16, isOutput=True)

    with tile.TileContext(nc) as tc, ExitStack() as ctx:
        wpool = ctx.enter_context(tc.tile_pool(name="w", bufs=1))
        apool = ctx.enter_context(tc.tile_pool(name="a", bufs=1))
        xpool = ctx.enter_context(tc.tile_pool(name="x", bufs=2))
        ypool = ctx.enter_context(tc.tile_pool(name="y", bufs=1))
        pspool = ctx.enter_context(tc.tile_pool(name="ps", bufs=PSUM_BUFS, space="PSUM"))
        spool = ctx.enter_context(tc.tile_pool(name="small", bufs=2))

        QW = OUT_FEATURES // 4
        OC_PER_Q = NOC // 4
        wt_src = wt_d[:, :].rearrange("(kt p) o -> p kt o", p=P)

        act_src = act_d[:, :].rearrange("p (s r) -> p s r", r=ROWS)

        def one_iter():
            # activations: one DMA per row-block (3D AP over the 2*KT chunks),
            # so iteration i+1's load of row-block rb only waits on iteration
            # i's matmuls that read rb — it starts ~3/4 of an iteration early
            a_t = apool.tile([P, 2 * KT * ROWS], bf16, tag="a", name="a_t")
            a_dst = a_t[:].rearrange("p (s r) -> p s r", r=ROWS)
            for rb in range(RB):
                csl = slice(rb * P, (rb + 1) * P)
                nc.sync.dma_start(out=a_dst[:, :, csl], in_=act_src[:, :, csl])

            # weights: one [128, KT*OUT_FEATURES] tile, kt-major columns;
            # 4 DMAs (one per outf quarter), each covering all KT k-tiles
            w_t = wpool.tile([P, KT * OUT_FEATURES], bf16, tag="w", name="w_t")
            wt_dst = w_t[:].rearrange("p (kt o) -> p kt o", kt=KT)
            for q in range(4):
                nc.sync.dma_start(
                    out=wt_dst[:, :, q * QW:(q + 1) * QW],
                    in_=wt_src[:, :, q * QW:(q + 1) * QW],
                )

            def lhs_ap(split, kt, rb):
                c = (split * KT + kt) * ROWS + rb * P
                return a_t[:, c:c + P]

            # The Matmult ISA struct has a single sync-wait slot, so matmuls
            # must never need both a DMA wait and a PSUM-WAR wait. Gate each
            # input DMA with a throwaway ldweights carrying the DMA wait.
            G_OC = GROUP_OC  # output chunks per stationary-reuse group
            for rb in range(RB):
                nc.tensor.ldweights(weights=a_t[:, rb * P:(rb + 1) * P])
                xt = xpool.tile([P, OUT_FEATURES], f32, tag="xt", name="xt")
                rsl = slice(rb * P, (rb + 1) * P)
                for og_start in range(0, NOC, G_OC):
                    ocs = range(og_start, min(og_start + G_OC, NOC))
                    if rb == 0:
                        # gate the weight-quarter DMAs this group first touches
                        for q in range(4):
                            if any(oc // OC_PER_Q == q for oc in ocs) and \
                               any(oc % OC_PER_Q == 0 for oc in ocs if oc // OC_PER_Q == q):
                                nc.tensor.ldweights(weights=w_t[:, q * QW:q * QW + P])
                    pts = [pspool.tile([P, 512], f32, tag="pt", name="pt")
                           for _ in ocs]
                    n = 0
                    for kt in range(KT):
                        for split in (0, 1):
                            for j, oc in enumerate(ocs):
                                nc.tensor.matmul(
                                    pts[j][:],
                                    lhsT=lhs_ap(split, kt, rb),
                                    rhs=w_t[:, kt * OUT_FEATURES + oc * 512:
                                            kt * OUT_FEATURES + (oc + 1) * 512],
                                    start=(n == 0),
                                    stop=(n == 2 * KT - 1),
                                )
                            n += 1
                    for j, oc in enumerate(ocs):
                        nc.scalar.copy(xt[:, oc * 512:(oc + 1) * 512], pts[j][:])

                yt = ypool.tile([P, OUT_FEATURES], bf16, tag="yt", name="yt")
                if PROBE == "nodve":
                    nc.vector.tensor_copy(yt[:, 0:P], xt[:, 0:P])
                else:
                    if PROBE == "noscan":
                        thresh = 5.0
                    else:
                        # segmented top-8 -> 256 candidates per row
                        cand = spool.tile([P, NSEG * 8], f32, tag="cand", name="cand")
                        for s in range(NSEG):
                            nc.vector.max(cand[:, 8 * s:8 * (s + 1)],
                                          xt[:, SEG * s:SEG * (s + 1)])
                        # peel 8 at a time to reach the k-th largest value
                        t8 = spool.tile([P, 8 * nrounds], f32, tag="t8", name="t8")
                        for r in range(nrounds):
                            nc.vector.max(t8[:, 8 * r:8 * (r + 1)], cand[:])
                            if r < nrounds - 1:
                                nc.vector.match_replace(
                                    cand[:], t8[:, 8 * r:8 * (r + 1)], cand[:], -1e30
                                )
                        ti = 8 * (nrounds - 1) + (k - 1) % 8
                        thresh = t8[:, ti:ti + 1]
                    # y = (x >= t) * x on DVE, in Y_SPLIT pieces so each
                    # piece's out-DMA overlaps the next piece's compute;
                    # separate bf16 tile keeps the out-DMA at one wait
                    # (the DMA pseudo-instruction has a single wait slot)
                    YW = OUT_FEATURES // Y_SPLIT
                    for yj in range(Y_SPLIT):
                        ysl = slice(yj * YW, (yj + 1) * YW)
                        nc.vector.scalar_tensor_tensor(
                            out=yt[:, ysl], in0=xt[:, ysl], scalar=thresh,
                            in1=xt[:, ysl],
                            op0=mybir.AluOpType.is_ge, op1=mybir.AluOpType.mult,
                        )
                eng = nc.gpsimd if OUT_DMA_ENGINE == "gpsimd" else nc.sync
                YW = OUT_FEATURES // Y_SPLIT
                for yj in range(Y_SPLIT):
                    ysl = slice(yj * YW, (yj + 1) * YW)
                    eng.dma_start(out=out_d[rsl, ysl], in_=yt[:, ysl])

        for _ in range(n_iter):
            one_iter()

    nc.compile()
    return nc


def _get_graph(k, n_iter=1):
    key = (k, n_iter, GROUP_OC, PSUM_BUFS, OUT_DMA_ENGINE, PROBE, Y_SPLIT)
    if key not in _graph_cache:
        _graph_cache[key] = _build(k, n_iter)
    return _graph_cache[key]


def _prep_weight(weight):
    w = np.asarray(weight, np.float32)
    key = (id(weight), w.shape,
           float(w[0, :16].sum()), float(w[-1, -16:].sum()), float(w.trace()))
    if key not in _w_cache:
        _w_cache.clear()
        wt = np.ascontiguousarray(w.T)
        _w_cache[key] = wt.astype(ml_dtypes.bfloat16)
    return _w_cache[key]


def _make_in_maps(input, weight):
    inp = np.asarray(input, np.float32)
    wt = _prep_weight(weight)
    inpT = np.ascontiguousarray(inp.T)            # [IN, BATCH]
    ah = inpT.astype(ml_dtypes.bfloat16)
    al = (inpT - ah.astype(np.float32)).astype(ml_dtypes.bfloat16)
    # pack [IN, BATCH] -> per-core [P, 2*KT*ROWS], col = split*KT*ROWS + kt*ROWS + r
    def pack(a, c):
        s = a[:, c * ROWS:(c + 1) * ROWS]                      # [IN, ROWS]
        return s.reshape(KT, P, ROWS).transpose(1, 0, 2).reshape(P, KT * ROWS)
    in_maps = []
    for c in range(N_CORES):
        in_maps.append({
            "act": np.ascontiguousarray(
                np.concatenate([pack(ah, c), pack(al, c)], axis=1)),
            "wt": wt,
        })
    return in_maps


def run_spmd(input, weight, hash_length, trace=False):
    """Run the SPMD kernel; returns (full_output, BassKernelResults)."""
    from concourse.bass_utils import run_bass_kernel_spmd
    k = int(hash_length)
    nc = _get_graph(k)
    in_maps = _make_in_maps(input, weight)
    res = run_bass_kernel_spmd(nc, in_maps, core_ids=list(range(N_CORES)), trace=trace)
    out = np.concatenate(
        [res.results[c]["out"].astype(np.float32) for c in range(N_CORES)], axis=0)
    return out, res


def kernel(input, weight, hash_length):
    out, _ = run_spmd(input, weight, hash_length, trace=False)
    return out


def make_bench_fn(input, weight, hash_length, n_iter):
    """Cached jitted shard_map over the n_iter-unrolled NEFF, with inputs
    uploaded once (not donated), for repeat-dispatch wall timing."""
    import jax
    import numpy as np_
    from jax.sharding import Mesh, PartitionSpec
    from jax.experimental.shard_map import shard_map
    from concourse import bass2jax
    import concourse.mybir as mybir

    bass2jax.install_neuronx_cc_hook()
    k = int(hash_length)
    nc = _get_graph(k, n_iter)
    in_maps = _make_in_maps(input, weight)

    part_name = nc.partition_id_tensor.name if nc.partition_id_tensor else None
    in_names, out_names, out_avals, zero_outs = [], [], [], []
    for alloc in nc.m.functions[0].allocations:
        if not isinstance(alloc, mybir.MemoryLocationSet):
            continue
        name = alloc.memorylocations[0].name
        if alloc.kind == "ExternalInput":
            if name != part_name:
                in_names.append(name)
        elif alloc.kind == "ExternalOutput":
            shape = tuple(alloc.tensor_shape)
            dtype = mybir.dt.np(alloc.dtype)
            out_names.append(name)
            out_avals.append(jax.core.ShapedArray(shape, dtype))
            zero_outs.append(np_.zeros((N_CORES * shape[0], *shape[1:]), dtype))
    n_params = len(in_names)
    all_names = in_names + out_names
    if part_name is not None:
        all_names = all_names + [part_name]

    def _body(*args):
        operands = list(args)
        if part_name is not None:
            operands.append(bass2jax.partition_id_tensor())
        outs = bass2jax._bass_exec_p.bind(
            *operands,
            out_avals=tuple(out_avals),
            in_names=tuple(all_names),
            out_names=tuple(out_names),
            lowering_input_output_aliases=(),
            sim_require_finite=True,
            sim_require_nnan=True,
            nc=nc,
        )
        return tuple(outs)

    devices = jax.devices()[:N_CORES]
    mesh = Mesh(np_.asarray(devices), ("core",))
    nin = n_params + len(out_names)
    fn = jax.jit(
        shard_map(_body, mesh=mesh,
                  in_specs=(PartitionSpec("core"),) * nin,
                  out_specs=(PartitionSpec("core"),) * len(out_names),
                  check_rep=False),
        keep_unused=True,
    )
    concat_in = [
        np_.concatenate([in_maps[c][nm] for c in range(N_CORES)], axis=0)
        for nm in in_names
    ]
    dev_args = [jax.device_put(a) for a in (*concat_in, *zero_outs)]
    jax.block_until_ready(dev_args)
    return fn, dev_args


# revision 46
# speedup vs baseline: 1.9695x; 1.0674x over previous
"""Top-k masking sparse projection on 8 TRN2 NeuronCores (Bass/Tile).

out = x * (x >= kth_largest_per_row(x)),  x = input @ weight.T
Data-parallel over the batch dim: each core handles 512 of 4096 rows.

Math: weight is binary (0/1), so splitting the input into bf16 hi+lo parts
makes both bf16 matmuls exact products; fp32 PSUM accumulation gives x to
~1e-6 abs, far below the typical spacing (~6e-3) between the 32nd/33rd
order statistics, so the kept set matches the fp32 reference.

Top-k per row (10240 wide, rows on partitions): 32 x max8 over contiguous
segments of 320 -> 256 candidates (a segment holding >8 of the row's top-32
has probability ~1e-6 per row); then 4 rounds of max8 + match_replace on the
candidates yield the exact 32nd-largest value; one fused
scalar_tensor_tensor pass applies (x >= t) * x in place.
"""

import numpy as np
import ml_dtypes
from contextlib import ExitStack

BATCH, IN_FEATURES, OUT_FEATURES, N_CORES = 4096, 512, 10240, 8
ROWS = BATCH // N_CORES          # rows per core
P = 128                          # partitions
RB = ROWS // P                   # row blocks per core
NOC = OUT_FEATURES // 512        # output chunks of 512
KT = IN_FEATURES // P            # contraction tiles
NSEG = 32                        # top-k candidate segments per row
SEG = OUT_FEATURES // NSEG       # segment length (320)

_graph_cache = {}
_w_cache = {}

# tuning knobs (affect graph build; bench sweeps override these)
GROUP_OC = 5      # output chunks sharing one stationary load (1 = no grouping)
PSUM_BUFS = 8
OUT_DMA_ENGINE = "gpsimd"  # "sync" | "gpsimd"
PROBE = ""        # "" | "noscan" (skip max8 scan+rounds) | "nodve" (skip scan+stt)
Y_SPLIT = 2       # pieces the masked bf16 output is written/DMA'd in
SCAN_SRC = "sbuf"  # "psum": max8 candidates straight from PSUM banks — measured
                   # ~35% SLOWER (DVE PSUM reads + bank contention); keep "sbuf"


def _build(k, n_iter=1):
    """Build the SPMD Bass graph for top-k threshold k (same on all cores).

    n_iter > 1 unrolls the whole computation (including input/weight DMA)
    back-to-back in one NEFF, for slope-based hardware timing.
    """
    import concourse.bass as bass
    import concourse.bacc as bacc
    import concourse.mybir as mybir
    from concourse import tile

    f32 = mybir.dt.float32
    bf16 = mybir.dt.bfloat16
    nrounds = (k + 7) // 8
    assert 1 <= k <= 64, f"unsupported hash_length {k}"

    nc = bacc.Bacc()
    # act: packed transposed activations, col = split*KT*ROWS + kt*ROWS + r
    act_d = nc.declare_dram_parameter("act", [P, 2 * KT * ROWS], bf16, isOutput=False)
    wt_d = nc.declare_dram_parameter("wt", [IN_FEATURES, OUT_FEATURES], bf16, isOutput=False)
    out_d = nc.declare_dram_parameter("out", [ROWS, OUT_FEATURES], bf16, isOutput=True)

    with tile.TileContext(nc) as tc, ExitStack() as ctx:
        wpool = ctx.enter_context(tc.tile_pool(name="w", bufs=1))
        apool = ctx.enter_context(tc.tile_pool(name="a", bufs=1))
        xpool = ctx.enter_context(tc.tile_pool(name="x", bufs=2))
        ypool = ctx.enter_context(tc.tile_pool(name="y", bufs=1))
        pspool = ctx.enter_context(tc.tile_pool(name="ps", bufs=PSUM_BUFS, space="PSUM"))
        spool = ctx.enter_context(tc.tile_pool(name="small", bufs=2))

        QW = OUT_FEATURES // 4
        OC_PER_Q = NOC // 4
        wt_src = wt_d[:, :].rearrange("(kt p) o -> p kt o", p=P)

        act_src = act_d[:, :].rearrange("p (s r) -> p s r", r=ROWS)

        def one_iter():
            # activations: one DMA per row-block (3D AP over the 2*KT chunks),
            # so iteration i+1's load of row-block rb only waits on iteration
            # i's matmuls that read rb — it starts ~3/4 of an iteration early
            a_t = apool.tile([P, 2 * KT * ROWS], bf16, tag="a", name="a_t")
            a_dst = a_t[:].rearrange("p (s r) -> p s r", r=ROWS)
            for rb in range(RB):
                csl = slice(rb * P, (rb + 1) * P)
                nc.sync.dma_start(out=a_dst[:, :, csl], in_=act_src[:, :, csl])

            # weights: one [128, KT*OUT_FEATURES] tile, kt-major columns;
            # 4 DMAs (one per outf quarter), each covering all KT k-tiles
            w_t = wpool.tile([P, KT * OUT_FEATURES], bf16, tag="w", name="w_t")
            wt_dst = w_t[:].rearrange("p (kt o) -> p kt o", kt=KT)
            for q in range(4):
                nc.sync.dma_start(
                    out=wt_dst[:, :, q * QW:(q + 1) * QW],
                    in_=wt_src[:, :, q * QW:(q + 1) * QW],
                )

            def lhs_ap(split, kt, rb):
                c = (split * KT + kt) * ROWS + rb * P
                return a_t[:, c:c + P]

            # The Matmult ISA struct has a single sync-wait slot, so matmuls
            # must never need both a DMA wait and a PSUM-WAR wait. Gate each
            # input DMA with a throwaway ldweights carrying the DMA wait.
            G_OC = GROUP_OC  # output chunks per stationary-reuse group
            for rb in range(RB):
                nc.tensor.ldweights(weights=a_t[:, rb * P:(rb + 1) * P])
                xt = xpool.tile([P, OUT_FEATURES], f32, tag="xt", name="xt")
                rsl = slice(rb * P, (rb + 1) * P)
                if SCAN_SRC == "psum" and not PROBE:
                    # 2 segments of 256 per PSUM bank -> 40 segs x top-8
                    candp = spool.tile([P, 2 * NOC * 8], f32, tag="cand",
                                       name="candp")
                for og_start in range(0, NOC, G_OC):
                    ocs = range(og_start, min(og_start + G_OC, NOC))
                    if rb == 0:
                        # gate the weight-quarter DMAs this group first touches
                        for q in range(4):
                            if any(oc // OC_PER_Q == q for oc in ocs) and \
                               any(oc % OC_PER_Q == 0 for oc in ocs if oc // OC_PER_Q == q):
                                nc.tensor.ldweights(weights=w_t[:, q * QW:q * QW + P])
                    pts = [pspool.tile([P, 512], f32, tag="pt", name="pt")
                           for _ in ocs]
                    n = 0
                    for kt in range(KT):
                        for split in (0, 1):
                            for j, oc in enumerate(ocs):
                                nc.tensor.matmul(
                                    pts[j][:],
                                    lhsT=lhs_ap(split, kt, rb),
                                    rhs=w_t[:, kt * OUT_FEATURES + oc * 512:
                                            kt * OUT_FEATURES + (oc + 1) * 512],
                                    start=(n == 0),
                                    stop=(n == 2 * KT - 1),
                                )
                            n += 1
                    for j, oc in enumerate(ocs):
                        nc.scalar.copy(xt[:, oc * 512:(oc + 1) * 512], pts[j][:])
                        if SCAN_SRC == "psum" and not PROBE:
                            nc.vector.max(candp[:, 16 * oc:16 * oc + 8],
                                          pts[j][:, 0:256])
                            nc.vector.max(candp[:, 16 * oc + 8:16 * oc + 16],
                                          pts[j][:, 256:512])

                yt = ypool.tile([P, OUT_FEATURES], bf16, tag="yt", name="yt")
                if PROBE == "nodve":
                    nc.vector.tensor_copy(yt[:, 0:P], xt[:, 0:P])
                else:
                    if PROBE == "noscan":
                        thresh = 5.0
                    else:
                        if SCAN_SRC == "psum":
                            cand = candp
                        else:
                            # segmented top-8 -> 256 candidates per row
                            cand = spool.tile([P, NSEG * 8], f32, tag="cand",
                                              name="cand")
                            for s in range(NSEG):
                                nc.vector.max(cand[:, 8 * s:8 * (s + 1)],
                                              xt[:, SEG * s:SEG * (s + 1)])
                        # peel 8 at a time to reach the k-th largest value
                        t8 = spool.tile([P, 8 * nrounds], f32, tag="t8", name="t8")
                        for r in range(nrounds):
                            nc.vector.max(t8[:, 8 * r:8 * (r + 1)], cand[:])
                            if r < nrounds - 1:
                                nc.vector.match_replace(
                                    cand[:], t8[:, 8 * r:8 * (r + 1)], cand[:], -1e30
                                )
                        ti = 8 * (nrounds - 1) + (k - 1) % 8
                        thresh = t8[:, ti:ti + 1]
                    # y = (x >= t) * x on DVE, in Y_SPLIT pieces so each
                    # piece's out-DMA overlaps the next piece's compute;
                    # separate bf16 tile keeps the out-DMA at one wait
                    # (the DMA pseudo-instruction has a single wait slot)
                    YW = OUT_FEATURES // Y_SPLIT
                    for yj in range(Y_SPLIT):
                        ysl = slice(yj * YW, (yj + 1) * YW)
                        nc.vector.scalar_tensor_tensor(
                            out=yt[:, ysl], in0=xt[:, ysl], scalar=thresh,
                            in1=xt[:, ysl],
                            op0=mybir.AluOpType.is_ge, op1=mybir.AluOpType.mult,
                        )
                eng = nc.gpsimd if OUT_DMA_ENGINE == "gpsimd" else nc.sync
                YW = OUT_FEATURES // Y_SPLIT
                for yj in range(Y_SPLIT):
                    ysl = slice(yj * YW, (yj + 1) * YW)
                    eng.dma_start(out=out_d[rsl, ysl], in_=yt[:, ysl])

        for _ in range(n_iter):
            one_iter()

    nc.compile()
    return nc


def _get_graph(k, n_iter=1):
    key = (k, n_iter, GROUP_OC, PSUM_BUFS, OUT_DMA_ENGINE, PROBE, Y_SPLIT, SCAN_SRC)
    if key not in _graph_cache:
        _graph_cache[key] = _build(k, n_iter)
    return _graph_cache[key]


def _prep_weight(weight):
    w = np.asarray(weight, np.float32)
    key = (id(weight), w.shape,
           float(w[0, :16].sum()), float(w[-1, -16:].sum()), float(w.trace()))
    if key not in _w_cache:
        _w_cache.clear()
        wt = np.ascontiguousarray(w.T)
        _w_cache[key] = wt.astype(ml_dtypes.bfloat16)
    return _w_cache[key]


def _make_in_maps(input, weight):
    inp = np.asarray(input, np.float32)
    wt = _prep_weight(weight)
    inpT = np.ascontiguousarray(inp.T)            # [IN, BATCH]
    ah = inpT.astype(ml_dtypes.bfloat16)
    al = (inpT - ah.astype(np.float32)).astype(ml_dtypes.bfloat16)
    # pack [IN, BATCH] -> per-core [P, 2*KT*ROWS], col = split*KT*ROWS + kt*ROWS + r
    def pack(a, c):
        s = a[:, c * ROWS:(c + 1) * ROWS]                      # [IN, ROWS]
        return s.reshape(KT, P, ROWS).transpose(1, 0, 2).reshape(P, KT * ROWS)
    in_maps = []
    for c in range(N_CORES):
        in_maps.append({
            "act": np.ascontiguousarray(
                np.concatenate([pack(ah, c), pack(al, c)], axis=1)),
            "wt": wt,
        })
    return in_maps


def run_spmd(input, weight, hash_length, trace=False):
    """Run the SPMD kernel; returns (full_output, BassKernelResults)."""
    from concourse.bass_utils import run_bass_kernel_spmd
    k = int(hash_length)
    nc = _get_graph(k)
    in_maps = _make_in_maps(input, weight)
    res = run_bass_kernel_spmd(nc, in_maps, core_ids=list(range(N_CORES)), trace=trace)
    out = np.concatenate(
        [res.results[c]["out"].astype(np.float32) for c in range(N_CORES)], axis=0)
    return out, res


def kernel(input, weight, hash_length):
    out, _ = run_spmd(input, weight, hash_length, trace=False)
    return out


def make_bench_fn(input, weight, hash_length, n_iter):
    """Cached jitted shard_map over the n_iter-unrolled NEFF, with inputs
    uploaded once (not donated), for repeat-dispatch wall timing."""
    import jax
    import numpy as np_
    from jax.sharding import Mesh, PartitionSpec
    from jax.experimental.shard_map import shard_map
    from concourse import bass2jax
    import concourse.mybir as mybir

    bass2jax.install_neuronx_cc_hook()
    k = int(hash_length)
    nc = _get_graph(k, n_iter)
    in_maps = _make_in_maps(input, weight)

    part_name = nc.partition_id_tensor.name if nc.partition_id_tensor else None
    in_names, out_names, out_avals, zero_outs = [], [], [], []
    for alloc in nc.m.functions[0].allocations:
        if not isinstance(alloc, mybir.MemoryLocationSet):
            continue
        name = alloc.memorylocations[0].name
        if alloc.kind == "ExternalInput":
            if name != part_name:
                in_names.append(name)
        elif alloc.kind == "ExternalOutput":
            shape = tuple(alloc.tensor_shape)
            dtype = mybir.dt.np(alloc.dtype)
            out_names.append(name)
            out_avals.append(jax.core.ShapedArray(shape, dtype))
            zero_outs.append(np_.zeros((N_CORES * shape[0], *shape[1:]), dtype))
    n_params = len(in_names)
    all_names = in_names + out_names
    if part_name is not None:
        all_names = all_names + [part_name]

    def _body(*args):
        operands = list(args)
        if part_name is not None:
            operands.append(bass2jax.partition_id_tensor())
        outs = bass2jax._bass_exec_p.bind(
            *operands,
            out_avals=tuple(out_avals),
            in_names=tuple(all_names),
            out_names=tuple(out_names),
            lowering_input_output_aliases=(),
            sim_require_finite=True,
            sim_require_nnan=True,
            nc=nc,
        )
        return tuple(outs)

    devices = jax.devices()[:N_CORES]
    mesh = Mesh(np_.asarray(devices), ("core",))
    nin = n_params + len(out_names)
    fn = jax.jit(
        shard_map(_body, mesh=mesh,
                  in_specs=(PartitionSpec("core"),) * nin,
                  out_specs=(PartitionSpec("core"),) * len(out_names),
                  check_rep=False),
        keep_unused=True,
    )
    concat_in = [
        np_.concatenate([in_maps[c][nm] for c in range(N_CORES)], axis=0)
        for nm in in_names
    ]
    dev_args = [jax.device_put(a) for a in (*concat_in, *zero_outs)]
    jax.block_until_ready(dev_args)
    return fn, dev_args


# revision 51
# speedup vs baseline: 2.0163x; 1.0238x over previous
"""Top-k masking sparse projection on 8 TRN2 NeuronCores (Bass/Tile).

out = x * (x >= kth_largest_per_row(x)),  x = input @ weight.T
Data-parallel over the batch dim: each core handles 512 of 4096 rows.

Math: weight is binary (0/1), so splitting the input into bf16 hi+lo parts
makes both bf16 matmuls exact products; fp32 PSUM accumulation gives x to
~1e-6 abs, far below the typical spacing (~6e-3) between the 32nd/33rd
order statistics, so the kept set matches the fp32 reference.

Top-k per row (10240 wide, rows on partitions): 32 x max8 over contiguous
segments of 320 -> 256 candidates (a segment holding >8 of the row's top-32
has probability ~1e-6 per row); then 4 rounds of max8 + match_replace on the
candidates yield the exact 32nd-largest value; one fused
scalar_tensor_tensor pass applies (x >= t) * x in place.
"""

import numpy as np
import ml_dtypes
from contextlib import ExitStack

BATCH, IN_FEATURES, OUT_FEATURES, N_CORES = 4096, 512, 10240, 8
ROWS = BATCH // N_CORES          # rows per core
P = 128                          # partitions
RB = ROWS // P                   # row blocks per core
NOC = OUT_FEATURES // 512        # output chunks of 512
KT = IN_FEATURES // P            # contraction tiles
NSEG = 32                        # top-k candidate segments per row
SEG = OUT_FEATURES // NSEG       # segment length (320)

_graph_cache = {}
_w_cache = {}

# tuning knobs (affect graph build; bench sweeps override these)
GROUP_OC = 5      # output chunks sharing one stationary load (1 = no grouping)
PSUM_BUFS = 8
OUT_DMA_ENGINE = "gpsimd"  # "sync" | "gpsimd"
PROBE = ""        # "" | "noscan" (skip max8 scan+rounds) | "nodve" (skip scan+stt)
Y_SPLIT = 2       # pieces the masked bf16 output is written/DMA'd in
SCAN_SRC = "sbuf"  # "psum": max8 candidates straight from PSUM banks — measured
                   # ~35% SLOWER (DVE PSUM reads + bank contention); keep "sbuf"
W_DTYPE = "fp8"    # "fp8": weight as e4m3 (0/1 exact) — halves weight SBUF+DMA,
                   # frees room for a 3rd x buffer; "bf16": fallback
XT_BUFS = 3


def _build(k, n_iter=1):
    """Build the SPMD Bass graph for top-k threshold k (same on all cores).

    n_iter > 1 unrolls the whole computation (including input/weight DMA)
    back-to-back in one NEFF, for slope-based hardware timing.
    """
    import concourse.bass as bass
    import concourse.bacc as bacc
    import concourse.mybir as mybir
    from concourse import tile

    f32 = mybir.dt.float32
    bf16 = mybir.dt.bfloat16
    wdt = mybir.dt.float8e4 if W_DTYPE == "fp8" else bf16
    nrounds = (k + 7) // 8
    assert 1 <= k <= 64, f"unsupported hash_length {k}"

    nc = bacc.Bacc()
    # act: packed transposed activations, col = split*KT*ROWS + kt*ROWS + r
    act_d = nc.declare_dram_parameter("act", [P, 2 * KT * ROWS], bf16, isOutput=False)
    wt_d = nc.declare_dram_parameter("wt", [IN_FEATURES, OUT_FEATURES], wdt, isOutput=False)
    out_d = nc.declare_dram_parameter("out", [ROWS, OUT_FEATURES], bf16, isOutput=True)

    with tile.TileContext(nc) as tc, ExitStack() as ctx:
        wpool = ctx.enter_context(tc.tile_pool(name="w", bufs=1))
        apool = ctx.enter_context(tc.tile_pool(name="a", bufs=1))
        xpool = ctx.enter_context(tc.tile_pool(name="x", bufs=XT_BUFS))
        ypool = ctx.enter_context(tc.tile_pool(name="y", bufs=1))
        pspool = ctx.enter_context(tc.tile_pool(name="ps", bufs=PSUM_BUFS, space="PSUM"))
        spool = ctx.enter_context(tc.tile_pool(name="small", bufs=2))

        QW = OUT_FEATURES // 4
        OC_PER_Q = NOC // 4
        wt_src = wt_d[:, :].rearrange("(kt p) o -> p kt o", p=P)

        act_src = act_d[:, :].rearrange("p (s r) -> p s r", r=ROWS)

        def one_iter():
            # activations: one DMA per row-block (3D AP over the 2*KT chunks),
            # so iteration i+1's load of row-block rb only waits on iteration
            # i's matmuls that read rb — it starts ~3/4 of an iteration early
            a_t = apool.tile([P, 2 * KT * ROWS], bf16, tag="a", name="a_t")
            a_dst = a_t[:].rearrange("p (s r) -> p s r", r=ROWS)
            for rb in range(RB):
                csl = slice(rb * P, (rb + 1) * P)
                nc.sync.dma_start(out=a_dst[:, :, csl], in_=act_src[:, :, csl])

            # weights: one [128, KT*OUT_FEATURES] tile, kt-major columns;
            # 4 DMAs (one per outf quarter), each covering all KT k-tiles
            w_t = wpool.tile([P, KT * OUT_FEATURES], wdt, tag="w", name="w_t")
            wt_dst = w_t[:].rearrange("p (kt o) -> p kt o", kt=KT)
            for q in range(4):
                nc.sync.dma_start(
                    out=wt_dst[:, :, q * QW:(q + 1) * QW],
                    in_=wt_src[:, :, q * QW:(q + 1) * QW],
                )

            def lhs_ap(split, kt, rb):
                c = (split * KT + kt) * ROWS + rb * P
                return a_t[:, c:c + P]

            # The Matmult ISA struct has a single sync-wait slot, so matmuls
            # must never need both a DMA wait and a PSUM-WAR wait. Gate each
            # input DMA with a throwaway ldweights carrying the DMA wait.
            G_OC = GROUP_OC  # output chunks per stationary-reuse group
            for rb in range(RB):
                nc.tensor.ldweights(weights=a_t[:, rb * P:(rb + 1) * P])
                xt = xpool.tile([P, OUT_FEATURES], f32, tag="xt", name="xt")
                rsl = slice(rb * P, (rb + 1) * P)
                if SCAN_SRC == "psum" and not PROBE:
                    # 2 segments of 256 per PSUM bank -> 40 segs x top-8
                    candp = spool.tile([P, 2 * NOC * 8], f32, tag="cand",
                                       name="candp")
                for og_start in range(0, NOC, G_OC):
                    ocs = range(og_start, min(og_start + G_OC, NOC))
                    if rb == 0:
                        # gate the weight-quarter DMAs this group first touches
                        for q in range(4):
                            if any(oc // OC_PER_Q == q for oc in ocs) and \
                               any(oc % OC_PER_Q == 0 for oc in ocs if oc // OC_PER_Q == q):
                                nc.tensor.ldweights(weights=w_t[:, q * QW:q * QW + P])
                    pts = [pspool.tile([P, 512], f32, tag="pt", name="pt")
                           for _ in ocs]
                    n = 0
                    for kt in range(KT):
                        for split in (0, 1):
                            for j, oc in enumerate(ocs):
                                nc.tensor.matmul(
                                    pts[j][:],
                                    lhsT=lhs_ap(split, kt, rb),
                                    rhs=w_t[:, kt * OUT_FEATURES + oc * 512:
                                            kt * OUT_FEATURES + (oc + 1) * 512],
                                    start=(n == 0),
                                    stop=(n == 2 * KT - 1),
                                )
                            n += 1
                    for j, oc in enumerate(ocs):
                        nc.scalar.copy(xt[:, oc * 512:(oc + 1) * 512], pts[j][:])
                        if SCAN_SRC == "psum" and not PROBE:
                            nc.vector.max(candp[:, 16 * oc:16 * oc + 8],
                                          pts[j][:, 0:256])
                            nc.vector.max(candp[:, 16 * oc + 8:16 * oc + 16],
                                          pts[j][:, 256:512])

                yt = ypool.tile([P, OUT_FEATURES], bf16, tag="yt", name="yt")
                if PROBE == "nodve":
                    nc.vector.tensor_copy(yt[:, 0:P], xt[:, 0:P])
                else:
                    if PROBE == "noscan":
                        thresh = 5.0
                    else:
                        if SCAN_SRC == "psum":
                            cand = candp
                        else:
                            # segmented top-8 -> 256 candidates per row
                            cand = spool.tile([P, NSEG * 8], f32, tag="cand",
                                              name="cand")
                            for s in range(NSEG):
                                nc.vector.max(cand[:, 8 * s:8 * (s + 1)],
                                              xt[:, SEG * s:SEG * (s + 1)])
                        # peel 8 at a time to reach the k-th largest value
                        t8 = spool.tile([P, 8 * nrounds], f32, tag="t8", name="t8")
                        for r in range(nrounds):
                            nc.vector.max(t8[:, 8 * r:8 * (r + 1)], cand[:])
                            if r < nrounds - 1:
                                nc.vector.match_replace(
                                    cand[:], t8[:, 8 * r:8 * (r + 1)], cand[:], -1e30
                                )
                        ti = 8 * (nrounds - 1) + (k - 1) % 8
                        thresh = t8[:, ti:ti + 1]
                    # y = (x >= t) * x on DVE, in Y_SPLIT pieces so each
                    # piece's out-DMA overlaps the next piece's compute;
                    # separate bf16 tile keeps the out-DMA at one wait
                    # (the DMA pseudo-instruction has a single wait slot)
                    YW = OUT_FEATURES // Y_SPLIT
                    for yj in range(Y_SPLIT):
                        ysl = slice(yj * YW, (yj + 1) * YW)
                        nc.vector.scalar_tensor_tensor(
                            out=yt[:, ysl], in0=xt[:, ysl], scalar=thresh,
                            in1=xt[:, ysl],
                            op0=mybir.AluOpType.is_ge, op1=mybir.AluOpType.mult,
                        )
                eng = nc.gpsimd if OUT_DMA_ENGINE == "gpsimd" else nc.sync
                YW = OUT_FEATURES // Y_SPLIT
                for yj in range(Y_SPLIT):
                    ysl = slice(yj * YW, (yj + 1) * YW)
                    eng.dma_start(out=out_d[rsl, ysl], in_=yt[:, ysl])

        for _ in range(n_iter):
            one_iter()

    nc.compile()
    return nc


def _get_graph(k, n_iter=1):
    key = (k, n_iter, GROUP_OC, PSUM_BUFS, OUT_DMA_ENGINE, PROBE, Y_SPLIT,
           SCAN_SRC, W_DTYPE, XT_BUFS)
    if key not in _graph_cache:
        _graph_cache[key] = _build(k, n_iter)
    return _graph_cache[key]


def _prep_weight(weight):
    w = np.asarray(weight, np.float32)
    wnp = ml_dtypes.float8_e4m3 if W_DTYPE == "fp8" else ml_dtypes.bfloat16
    key = (id(weight), w.shape, W_DTYPE,
           float(w[0, :16].sum()), float(w[-1, -16:].sum()), float(w.trace()))
    if key not in _w_cache:
        _w_cache.clear()
        wt = np.ascontiguousarray(w.T)
        _w_cache[key] = wt.astype(wnp)
    return _w_cache[key]


def _make_in_maps(input, weight):
    inp = np.asarray(input, np.float32)
    wt = _prep_weight(weight)
    inpT = np.ascontiguousarray(inp.T)            # [IN, BATCH]
    ah = inpT.astype(ml_dtypes.bfloat16)
    al = (inpT - ah.astype(np.float32)).astype(ml_dtypes.bfloat16)
    # pack [IN, BATCH] -> per-core [P, 2*KT*ROWS], col = split*KT*ROWS + kt*ROWS + r
    def pack(a, c):
        s = a[:, c * ROWS:(c + 1) * ROWS]                      # [IN, ROWS]
        return s.reshape(KT, P, ROWS).transpose(1, 0, 2).reshape(P, KT * ROWS)
    in_maps = []
    for c in range(N_CORES):
        in_maps.append({
            "act": np.ascontiguousarray(
                np.concatenate([pack(ah, c), pack(al, c)], axis=1)),
            "wt": wt,
        })
    return in_maps


def run_spmd(input, weight, hash_length, trace=False):
    """Run the SPMD kernel; returns (full_output, BassKernelResults)."""
    from concourse.bass_utils import run_bass_kernel_spmd
    k = int(hash_length)
    nc = _get_graph(k)
    in_maps = _make_in_maps(input, weight)
    res = run_bass_kernel_spmd(nc, in_maps, core_ids=list(range(N_CORES)), trace=trace)
    out = np.concatenate(
        [res.results[c]["out"].astype(np.float32) for c in range(N_CORES)], axis=0)
    return out, res


def kernel(input, weight, hash_length):
    out, _ = run_spmd(input, weight, hash_length, trace=False)
    return out


def make_bench_fn(input, weight, hash_length, n_iter):
    """Cached jitted shard_map over the n_iter-unrolled NEFF, with inputs
    uploaded once (not donated), for repeat-dispatch wall timing."""
    import jax
    import numpy as np_
    from jax.sharding import Mesh, PartitionSpec
    from jax.experimental.shard_map import shard_map
    from concourse import bass2jax
    import concourse.mybir as mybir

    bass2jax.install_neuronx_cc_hook()
    k = int(hash_length)
    nc = _get_graph(k, n_iter)
    in_maps = _make_in_maps(input, weight)

    part_name = nc.partition_id_tensor.name if nc.partition_id_tensor else None
    in_names, out_names, out_avals, zero_outs = [], [], [], []
    for alloc in nc.m.functions[0].allocations:
        if not isinstance(alloc, mybir.MemoryLocationSet):
            continue
        name = alloc.memorylocations[0].name
        if alloc.kind == "ExternalInput":
            if name != part_name:
                in_names.append(name)
        elif alloc.kind == "ExternalOutput":
            shape = tuple(alloc.tensor_shape)
            dtype = mybir.dt.np(alloc.dtype)
            out_names.append(name)
            out_avals.append(jax.core.ShapedArray(shape, dtype))
            zero_outs.append(np_.zeros((N_CORES * shape[0], *shape[1:]), dtype))
    n_params = len(in_names)
    all_names = in_names + out_names
    if part_name is not None:
        all_names = all_names + [part_name]

    def _body(*args):
        operands = list(args)
        if part_name is not None:
            operands.append(bass2jax.partition_id_tensor())
        outs = bass2jax._bass_exec_p.bind(
            *operands,
            out_avals=tuple(out_avals),
            in_names=tuple(all_names),
            out_names=tuple(out_names),
            lowering_input_output_aliases=(),
            sim_require_finite=True,
            sim_require_nnan=True,
            nc=nc,
        )
        return tuple(outs)

    devices = jax.devices()[:N_CORES]
    mesh = Mesh(np_.asarray(devices), ("core",))
    nin = n_params + len(out_names)
    fn = jax.jit(
        shard_map(_body, mesh=mesh,
                  in_specs=(PartitionSpec("core"),) * nin,
                  out_specs=(PartitionSpec("core"),) * len(out_names),
                  check_rep=False),
        keep_unused=True,
    )
    concat_in = [
        np_.concatenate([in_maps[c][nm] for c in range(N_CORES)], axis=0)
        for nm in in_names
    ]
    dev_args = [jax.device_put(a) for a in (*concat_in, *zero_outs)]
    jax.block_until_ready(dev_args)
    return fn, dev_args


# revision 55
# speedup vs baseline: 2.0784x; 1.0308x over previous
"""Top-k masking sparse projection on 8 TRN2 NeuronCores (Bass/Tile).

out = x * (x >= kth_largest_per_row(x)),  x = input @ weight.T
Data-parallel over the batch dim: each core handles 512 of 4096 rows.

Math: weight is binary (0/1), so splitting the input into bf16 hi+lo parts
makes both bf16 matmuls exact products; fp32 PSUM accumulation gives x to
~1e-6 abs, far below the typical spacing (~6e-3) between the 32nd/33rd
order statistics, so the kept set matches the fp32 reference.

Top-k per row (10240 wide, rows on partitions): 32 x max8 over contiguous
segments of 320 -> 256 candidates (a segment holding >8 of the row's top-32
has probability ~1e-6 per row); then 4 rounds of max8 + match_replace on the
candidates yield the exact 32nd-largest value; one fused
scalar_tensor_tensor pass applies (x >= t) * x in place.
"""

import numpy as np
import ml_dtypes
from contextlib import ExitStack

BATCH, IN_FEATURES, OUT_FEATURES, N_CORES = 4096, 512, 10240, 8
ROWS = BATCH // N_CORES          # rows per core
P = 128                          # partitions
RB = ROWS // P                   # row blocks per core
NOC = OUT_FEATURES // 512        # output chunks of 512
KT = IN_FEATURES // P            # contraction tiles
NSEG = 32                        # top-k candidate segments per row
SEG = OUT_FEATURES // NSEG       # segment length (320)

_graph_cache = {}
_w_cache = {}

# tuning knobs (affect graph build; bench sweeps override these)
GROUP_OC = 5      # output chunks sharing one stationary load (1 = no grouping)
PSUM_BUFS = 8
OUT_DMA_ENGINE = "gpsimd"  # "sync" | "gpsimd"
PROBE = ""        # "" | "noscan" (skip max8 scan+rounds) | "nodve" (skip scan+stt)
Y_SPLIT = 2       # pieces the masked bf16 output is written/DMA'd in
SCAN_SRC = "sbuf"  # "psum": max8 candidates straight from PSUM banks — measured
                   # ~35% SLOWER (DVE PSUM reads + bank contention); keep "sbuf"
W_DTYPE = "fp8"    # "fp8": weight as e4m3 (0/1 exact) — halves weight SBUF+DMA,
                   # frees room for a 3rd x buffer; "bf16": fallback
XT_BUFS = 2       # x2/a2/y2 (fits in 179KB with fp8 weights) measured best:
A_BUFS = 2        # double-buffered act + masked-out tiles remove both
Y_BUFS = 2        # iteration-boundary WAR serializations


def _build(k, n_iter=1):
    """Build the SPMD Bass graph for top-k threshold k (same on all cores).

    n_iter > 1 unrolls the whole computation (including input/weight DMA)
    back-to-back in one NEFF, for slope-based hardware timing.
    """
    import concourse.bass as bass
    import concourse.bacc as bacc
    import concourse.mybir as mybir
    from concourse import tile

    f32 = mybir.dt.float32
    bf16 = mybir.dt.bfloat16
    wdt = mybir.dt.float8e4 if W_DTYPE == "fp8" else bf16
    nrounds = (k + 7) // 8
    assert 1 <= k <= 64, f"unsupported hash_length {k}"

    nc = bacc.Bacc()
    # act: packed transposed activations, col = split*KT*ROWS + kt*ROWS + r
    act_d = nc.declare_dram_parameter("act", [P, 2 * KT * ROWS], bf16, isOutput=False)
    wt_d = nc.declare_dram_parameter("wt", [IN_FEATURES, OUT_FEATURES], wdt, isOutput=False)
    out_d = nc.declare_dram_parameter("out", [ROWS, OUT_FEATURES], bf16, isOutput=True)

    with tile.TileContext(nc) as tc, ExitStack() as ctx:
        wpool = ctx.enter_context(tc.tile_pool(name="w", bufs=1))
        apool = ctx.enter_context(tc.tile_pool(name="a", bufs=A_BUFS))
        xpool = ctx.enter_context(tc.tile_pool(name="x", bufs=XT_BUFS))
        ypool = ctx.enter_context(tc.tile_pool(name="y", bufs=Y_BUFS))
        pspool = ctx.enter_context(tc.tile_pool(name="ps", bufs=PSUM_BUFS, space="PSUM"))
        spool = ctx.enter_context(tc.tile_pool(name="small", bufs=2))

        QW = OUT_FEATURES // 4
        OC_PER_Q = NOC // 4
        wt_src = wt_d[:, :].rearrange("(kt p) o -> p kt o", p=P)

        act_src = act_d[:, :].rearrange("p (s r) -> p s r", r=ROWS)

        def one_iter():
            # activations: one DMA per row-block (3D AP over the 2*KT chunks),
            # so iteration i+1's load of row-block rb only waits on iteration
            # i's matmuls that read rb — it starts ~3/4 of an iteration early
            a_t = apool.tile([P, 2 * KT * ROWS], bf16, tag="a", name="a_t")
            a_dst = a_t[:].rearrange("p (s r) -> p s r", r=ROWS)
            for rb in range(RB):
                csl = slice(rb * P, (rb + 1) * P)
                nc.sync.dma_start(out=a_dst[:, :, csl], in_=act_src[:, :, csl])

            # weights: one [128, KT*OUT_FEATURES] tile, kt-major columns;
            # 4 DMAs (one per outf quarter), each covering all KT k-tiles
            w_t = wpool.tile([P, KT * OUT_FEATURES], wdt, tag="w", name="w_t")
            wt_dst = w_t[:].rearrange("p (kt o) -> p kt o", kt=KT)
            for q in range(4):
                nc.sync.dma_start(
                    out=wt_dst[:, :, q * QW:(q + 1) * QW],
                    in_=wt_src[:, :, q * QW:(q + 1) * QW],
                )

            def lhs_ap(split, kt, rb):
                c = (split * KT + kt) * ROWS + rb * P
                return a_t[:, c:c + P]

            # The Matmult ISA struct has a single sync-wait slot, so matmuls
            # must never need both a DMA wait and a PSUM-WAR wait. Gate each
            # input DMA with a throwaway ldweights carrying the DMA wait.
            G_OC = GROUP_OC  # output chunks per stationary-reuse group
            for rb in range(RB):
                nc.tensor.ldweights(weights=a_t[:, rb * P:(rb + 1) * P])
                xt = xpool.tile([P, OUT_FEATURES], f32, tag="xt", name="xt")
                rsl = slice(rb * P, (rb + 1) * P)
                if SCAN_SRC == "psum" and not PROBE:
                    # 2 segments of 256 per PSUM bank -> 40 segs x top-8
                    candp = spool.tile([P, 2 * NOC * 8], f32, tag="cand",
                                       name="candp")
                for og_start in range(0, NOC, G_OC):
                    ocs = range(og_start, min(og_start + G_OC, NOC))
                    if rb == 0:
                        # gate the weight-quarter DMAs this group first touches
                        for q in range(4):
                            if any(oc // OC_PER_Q == q for oc in ocs) and \
                               any(oc % OC_PER_Q == 0 for oc in ocs if oc // OC_PER_Q == q):
                                nc.tensor.ldweights(weights=w_t[:, q * QW:q * QW + P])
                    pts = [pspool.tile([P, 512], f32, tag="pt", name="pt")
                           for _ in ocs]
                    n = 0
                    for kt in range(KT):
                        for split in (0, 1):
                            for j, oc in enumerate(ocs):
                                nc.tensor.matmul(
                                    pts[j][:],
                                    lhsT=lhs_ap(split, kt, rb),
                                    rhs=w_t[:, kt * OUT_FEATURES + oc * 512:
                                            kt * OUT_FEATURES + (oc + 1) * 512],
                                    start=(n == 0),
                                    stop=(n == 2 * KT - 1),
                                )
                            n += 1
                    for j, oc in enumerate(ocs):
                        nc.scalar.copy(xt[:, oc * 512:(oc + 1) * 512], pts[j][:])
                        if SCAN_SRC == "psum" and not PROBE:
                            nc.vector.max(candp[:, 16 * oc:16 * oc + 8],
                                          pts[j][:, 0:256])
                            nc.vector.max(candp[:, 16 * oc + 8:16 * oc + 16],
                                          pts[j][:, 256:512])

                yt = ypool.tile([P, OUT_FEATURES], bf16, tag="yt", name="yt")
                if PROBE == "nodve":
                    nc.vector.tensor_copy(yt[:, 0:P], xt[:, 0:P])
                else:
                    if PROBE == "noscan":
                        thresh = 5.0
                    else:
                        if SCAN_SRC == "psum":
                            cand = candp
                        else:
                            # segmented top-8 -> 256 candidates per row
                            cand = spool.tile([P, NSEG * 8], f32, tag="cand",
                                              name="cand")
                            for s in range(NSEG):
                                nc.vector.max(cand[:, 8 * s:8 * (s + 1)],
                                              xt[:, SEG * s:SEG * (s + 1)])
                        # peel 8 at a time to reach the k-th largest value
                        t8 = spool.tile([P, 8 * nrounds], f32, tag="t8", name="t8")
                        for r in range(nrounds):
                            nc.vector.max(t8[:, 8 * r:8 * (r + 1)], cand[:])
                            if r < nrounds - 1:
                                nc.vector.match_replace(
                                    cand[:], t8[:, 8 * r:8 * (r + 1)], cand[:], -1e30
                                )
                        ti = 8 * (nrounds - 1) + (k - 1) % 8
                        thresh = t8[:, ti:ti + 1]
                    # y = (x >= t) * x on DVE, in Y_SPLIT pieces so each
                    # piece's out-DMA overlaps the next piece's compute;
                    # separate bf16 tile keeps the out-DMA at one wait
                    # (the DMA pseudo-instruction has a single wait slot)
                    YW = OUT_FEATURES // Y_SPLIT
                    for yj in range(Y_SPLIT):
                        ysl = slice(yj * YW, (yj + 1) * YW)
                        nc.vector.scalar_tensor_tensor(
                            out=yt[:, ysl], in0=xt[:, ysl], scalar=thresh,
                            in1=xt[:, ysl],
                            op0=mybir.AluOpType.is_ge, op1=mybir.AluOpType.mult,
                        )
                eng = nc.gpsimd if OUT_DMA_ENGINE == "gpsimd" else nc.sync
                YW = OUT_FEATURES // Y_SPLIT
                for yj in range(Y_SPLIT):
                    ysl = slice(yj * YW, (yj + 1) * YW)
                    eng.dma_start(out=out_d[rsl, ysl], in_=yt[:, ysl])

        for _ in range(n_iter):
            one_iter()

    nc.compile()
    return nc


def _get_graph(k, n_iter=1):
    key = (k, n_iter, GROUP_OC, PSUM_BUFS, OUT_DMA_ENGINE, PROBE, Y_SPLIT,
           SCAN_SRC, W_DTYPE, XT_BUFS, A_BUFS, Y_BUFS)
    if key not in _graph_cache:
        _graph_cache[key] = _build(k, n_iter)
    return _graph_cache[key]


def _prep_weight(weight):
    w = np.asarray(weight, np.float32)
    wnp = ml_dtypes.float8_e4m3 if W_DTYPE == "fp8" else ml_dtypes.bfloat16
    key = (id(weight), w.shape, W_DTYPE,
           float(w[0, :16].sum()), float(w[-1, -16:].sum()), float(w.trace()))
    if key not in _w_cache:
        _w_cache.clear()
        wt = np.ascontiguousarray(w.T)
        _w_cache[key] = wt.astype(wnp)
    return _w_cache[key]


def _make_in_maps(input, weight):
    inp = np.asarray(input, np.float32)
    wt = _prep_weight(weight)
    inpT = np.ascontiguousarray(inp.T)            # [IN, BATCH]
    ah = inpT.astype(ml_dtypes.bfloat16)
    al = (inpT - ah.astype(np.float32)).astype(ml_dtypes.bfloat16)
    # pack [IN, BATCH] -> per-core [P, 2*KT*ROWS], col = split*KT*ROWS + kt*ROWS + r
    def pack(a, c):
        s = a[:, c * ROWS:(c + 1) * ROWS]                      # [IN, ROWS]
        return s.reshape(KT, P, ROWS).transpose(1, 0, 2).reshape(P, KT * ROWS)
    in_maps = []
    for c in range(N_CORES):
        in_maps.append({
            "act": np.ascontiguousarray(
                np.concatenate([pack(ah, c), pack(al, c)], axis=1)),
            "wt": wt,
        })
    return in_maps


def run_spmd(input, weight, hash_length, trace=False):
    """Run the SPMD kernel; returns (full_output, BassKernelResults)."""
    from concourse.bass_utils import run_bass_kernel_spmd
    k = int(hash_length)
    nc = _get_graph(k)
    in_maps = _make_in_maps(input, weight)
    res = run_bass_kernel_spmd(nc, in_maps, core_ids=list(range(N_CORES)), trace=trace)
    out = np.concatenate(
        [res.results[c]["out"].astype(np.float32) for c in range(N_CORES)], axis=0)
    return out, res


def kernel(input, weight, hash_length):
    out, _ = run_spmd(input, weight, hash_length, trace=False)
    return out


def make_bench_fn(input, weight, hash_length, n_iter):
    """Cached jitted shard_map over the n_iter-unrolled NEFF, with inputs
    uploaded once (not donated), for repeat-dispatch wall timing."""
    import jax
    import numpy as np_
    from jax.sharding import Mesh, PartitionSpec
    from jax.experimental.shard_map import shard_map
    from concourse import bass2jax
    import concourse.mybir as mybir

    bass2jax.install_neuronx_cc_hook()
    k = int(hash_length)
    nc = _get_graph(k, n_iter)
    in_maps = _make_in_maps(input, weight)

    part_name = nc.partition_id_tensor.name if nc.partition_id_tensor else None
    in_names, out_names, out_avals, zero_outs = [], [], [], []
    for alloc in nc.m.functions[0].allocations:
        if not isinstance(alloc, mybir.MemoryLocationSet):
            continue
        name = alloc.memorylocations[0].name
        if alloc.kind == "ExternalInput":
            if name != part_name:
                in_names.append(name)
        elif alloc.kind == "ExternalOutput":
            shape = tuple(alloc.tensor_shape)
            dtype = mybir.dt.np(alloc.dtype)
            out_names.append(name)
            out_avals.append(jax.core.ShapedArray(shape, dtype))
            zero_outs.append(np_.zeros((N_CORES * shape[0], *shape[1:]), dtype))
    n_params = len(in_names)
    all_names = in_names + out_names
    if part_name is not None:
        all_names = all_names + [part_name]

    def _body(*args):
        operands = list(args)
        if part_name is not None:
            operands.append(bass2jax.partition_id_tensor())
        outs = bass2jax._bass_exec_p.bind(
            *operands,
            out_avals=tuple(out_avals),
            in_names=tuple(all_names),
            out_names=tuple(out_names),
            lowering_input_output_aliases=(),
            sim_require_finite=True,
            sim_require_nnan=True,
            nc=nc,
        )
        return tuple(outs)

    devices = jax.devices()[:N_CORES]
    mesh = Mesh(np_.asarray(devices), ("core",))
    nin = n_params + len(out_names)
    fn = jax.jit(
        shard_map(_body, mesh=mesh,
                  in_specs=(PartitionSpec("core"),) * nin,
                  out_specs=(PartitionSpec("core"),) * len(out_names),
                  check_rep=False),
        keep_unused=True,
    )
    concat_in = [
        np_.concatenate([in_maps[c][nm] for c in range(N_CORES)], axis=0)
        for nm in in_names
    ]
    dev_args = [jax.device_put(a) for a in (*concat_in, *zero_outs)]
    jax.block_until_ready(dev_args)
    return fn, dev_args


# revision 56
# speedup vs baseline: 2.2727x; 1.0935x over previous
"""Top-k masking sparse projection on 8 TRN2 NeuronCores (Bass/Tile).

out = x * (x >= kth_largest_per_row(x)),  x = input @ weight.T
Data-parallel over the batch dim: each core handles 512 of 4096 rows.

Math: weight is binary (0/1), so splitting the input into bf16 hi+lo parts
makes both bf16 matmuls exact products; fp32 PSUM accumulation gives x to
~1e-6 abs, far below the typical spacing (~6e-3) between the 32nd/33rd
order statistics, so the kept set matches the fp32 reference.

Top-k per row (10240 wide, rows on partitions): 32 x max8 over contiguous
segments of 320 -> 256 candidates (a segment holding >8 of the row's top-32
has probability ~1e-6 per row); then 4 rounds of max8 + match_replace on the
candidates yield the exact 32nd-largest value; one fused
scalar_tensor_tensor pass applies (x >= t) * x in place.
"""

import numpy as np
import ml_dtypes
from contextlib import ExitStack

BATCH, IN_FEATURES, OUT_FEATURES, N_CORES = 4096, 512, 10240, 8
ROWS = BATCH // N_CORES          # rows per core
P = 128                          # partitions
RB = ROWS // P                   # row blocks per core
NOC = OUT_FEATURES // 512        # output chunks of 512
KT = IN_FEATURES // P            # contraction tiles
NSEG = 32                        # top-k candidate segments per row
SEG = OUT_FEATURES // NSEG       # segment length (320)

_graph_cache = {}
_w_cache = {}

# tuning knobs (affect graph build; bench sweeps override these)
GROUP_OC = 5      # output chunks sharing one stationary load (1 = no grouping)
PSUM_BUFS = 8
OUT_DMA_ENGINE = "gpsimd"  # "sync" | "gpsimd"
PROBE = ""        # "" | "noscan" (skip max8 scan+rounds) | "nodve" (skip scan+stt)
Y_SPLIT = 2       # pieces the masked bf16 output is written/DMA'd in
SCAN_SRC = "sbuf"  # "psum": max8 candidates straight from PSUM banks — measured
                   # ~35% SLOWER (DVE PSUM reads + bank contention); keep "sbuf"
W_DTYPE = "fp8"    # "fp8": weight as e4m3 (0/1 exact) — halves weight SBUF+DMA,
                   # frees room for a 3rd x buffer; "bf16": fallback
XT_BUFS = 2       # x2/a2/y2 (fits in 179KB with fp8 weights) measured best:
A_BUFS = 2        # double-buffered act + masked-out tiles remove both
Y_BUFS = 2        # iteration-boundary WAR serializations


def _build(k, n_iter=1):
    """Build the SPMD Bass graph for top-k threshold k (same on all cores).

    n_iter > 1 unrolls the whole computation (including input/weight DMA)
    back-to-back in one NEFF, for slope-based hardware timing.
    """
    import concourse.bass as bass
    import concourse.bacc as bacc
    import concourse.mybir as mybir
    from concourse import tile

    f32 = mybir.dt.float32
    bf16 = mybir.dt.bfloat16
    wdt = mybir.dt.float8e4 if W_DTYPE == "fp8" else bf16
    nrounds = (k + 7) // 8
    assert 1 <= k <= 64, f"unsupported hash_length {k}"

    nc = bacc.Bacc()
    # act: packed transposed activations, col = split*KT*ROWS + kt*ROWS + r
    act_d = nc.declare_dram_parameter("act", [P, 2 * KT * ROWS], bf16, isOutput=False)
    wt_d = nc.declare_dram_parameter("wt", [IN_FEATURES, OUT_FEATURES], wdt, isOutput=False)
    out_d = nc.declare_dram_parameter("out", [ROWS, OUT_FEATURES], bf16, isOutput=True)

    with tile.TileContext(nc) as tc, ExitStack() as ctx:
        wpool = ctx.enter_context(tc.tile_pool(name="w", bufs=1))
        apool = ctx.enter_context(tc.tile_pool(name="a", bufs=A_BUFS))
        xpool = ctx.enter_context(tc.tile_pool(name="x", bufs=XT_BUFS))
        ypool = ctx.enter_context(tc.tile_pool(name="y", bufs=Y_BUFS))
        pspool = ctx.enter_context(tc.tile_pool(name="ps", bufs=PSUM_BUFS, space="PSUM"))
        spool = ctx.enter_context(tc.tile_pool(name="small", bufs=2))

        QW = OUT_FEATURES // 4
        OC_PER_Q = NOC // 4
        wt_src = wt_d[:, :].rearrange("(kt p) o -> p kt o", p=P)

        act_src = act_d[:, :].rearrange("p (s r) -> p s r", r=ROWS)

        def one_iter():
            # activations: one DMA per row-block (3D AP over the 2*KT chunks),
            # so iteration i+1's load of row-block rb only waits on iteration
            # i's matmuls that read rb — it starts ~3/4 of an iteration early
            a_t = apool.tile([P, 2 * KT * ROWS], bf16, tag="a", name="a_t")
            a_dst = a_t[:].rearrange("p (s r) -> p s r", r=ROWS)
            for rb in range(RB):
                csl = slice(rb * P, (rb + 1) * P)
                nc.sync.dma_start(out=a_dst[:, :, csl], in_=act_src[:, :, csl])

            # weights: one [128, KT*OUT_FEATURES] tile, kt-major columns;
            # 4 DMAs (one per outf quarter), each covering all KT k-tiles
            w_t = wpool.tile([P, KT * OUT_FEATURES], wdt, tag="w", name="w_t")
            wt_dst = w_t[:].rearrange("p (kt o) -> p kt o", kt=KT)
            for q in range(4):
                nc.sync.dma_start(
                    out=wt_dst[:, :, q * QW:(q + 1) * QW],
                    in_=wt_src[:, :, q * QW:(q + 1) * QW],
                )

            def lhs_ap(split, kt, rb):
                c = (split * KT + kt) * ROWS + rb * P
                return a_t[:, c:c + P]

            # The Matmult ISA struct has a single sync-wait slot, so matmuls
            # must never need both a DMA wait and a PSUM-WAR wait. Gate each
            # input DMA with a throwaway ldweights carrying the DMA wait.
            G_OC = GROUP_OC  # output chunks per stationary-reuse group
            for rb in range(RB):
                nc.tensor.ldweights(weights=a_t[:, rb * P:(rb + 1) * P])
                xt = xpool.tile([P, OUT_FEATURES], f32, tag="xt", name="xt")
                rsl = slice(rb * P, (rb + 1) * P)
                if SCAN_SRC == "psum" and not PROBE:
                    # 2 segments of 256 per PSUM bank -> 40 segs x top-8
                    candp = spool.tile([P, 2 * NOC * 8], f32, tag="cand",
                                       name="candp")
                for og_start in range(0, NOC, G_OC):
                    ocs = range(og_start, min(og_start + G_OC, NOC))
                    if rb == 0:
                        # gate the weight-quarter DMAs this group first touches
                        for q in range(4):
                            if any(oc // OC_PER_Q == q for oc in ocs) and \
                               any(oc % OC_PER_Q == 0 for oc in ocs if oc // OC_PER_Q == q):
                                nc.tensor.ldweights(weights=w_t[:, q * QW:q * QW + P])
                    pts = [pspool.tile([P, 512], f32, tag="pt", name="pt")
                           for _ in ocs]
                    n = 0
                    for kt in range(KT):
                        for split in (0, 1):
                            for j, oc in enumerate(ocs):
                                nc.tensor.matmul(
                                    pts[j][:],
                                    lhsT=lhs_ap(split, kt, rb),
                                    rhs=w_t[:, kt * OUT_FEATURES + oc * 512:
                                            kt * OUT_FEATURES + (oc + 1) * 512],
                                    start=(n == 0),
                                    stop=(n == 2 * KT - 1),
                                )
                            n += 1
                    for j, oc in enumerate(ocs):
                        nc.scalar.copy(xt[:, oc * 512:(oc + 1) * 512], pts[j][:])
                        if SCAN_SRC == "psum" and not PROBE:
                            nc.vector.max(candp[:, 16 * oc:16 * oc + 8],
                                          pts[j][:, 0:256])
                            nc.vector.max(candp[:, 16 * oc + 8:16 * oc + 16],
                                          pts[j][:, 256:512])

                yt = ypool.tile([P, OUT_FEATURES], bf16, tag="yt", name="yt")
                if PROBE == "nodve":
                    nc.vector.tensor_copy(yt[:, 0:P], xt[:, 0:P])
                else:
                    if PROBE == "noscan":
                        thresh = 5.0
                    else:
                        if SCAN_SRC == "psum":
                            cand = candp
                        else:
                            # segmented top-8 -> 256 candidates per row
                            cand = spool.tile([P, NSEG * 8], f32, tag="cand",
                                              name="cand")
                            for s in range(NSEG):
                                nc.vector.max(cand[:, 8 * s:8 * (s + 1)],
                                              xt[:, SEG * s:SEG * (s + 1)])
                        # peel 8 at a time to reach the k-th largest value
                        t8 = spool.tile([P, 8 * nrounds], f32, tag="t8", name="t8")
                        for r in range(nrounds):
                            nc.vector.max(t8[:, 8 * r:8 * (r + 1)], cand[:])
                            if r < nrounds - 1:
                                nc.vector.match_replace(
                                    cand[:], t8[:, 8 * r:8 * (r + 1)], cand[:], -1e30
                                )
                        ti = 8 * (nrounds - 1) + (k - 1) % 8
                        thresh = t8[:, ti:ti + 1]
                    # y = (x >= t) * x on DVE, in Y_SPLIT pieces so each
                    # piece's out-DMA overlaps the next piece's compute;
                    # separate bf16 tile keeps the out-DMA at one wait
                    # (the DMA pseudo-instruction has a single wait slot)
                    YW = OUT_FEATURES // Y_SPLIT
                    for yj in range(Y_SPLIT):
                        ysl = slice(yj * YW, (yj + 1) * YW)
                        nc.vector.scalar_tensor_tensor(
                            out=yt[:, ysl], in0=xt[:, ysl], scalar=thresh,
                            in1=xt[:, ysl],
                            op0=mybir.AluOpType.is_ge, op1=mybir.AluOpType.mult,
                        )
                YW = OUT_FEATURES // Y_SPLIT
                for yj in range(Y_SPLIT):
                    if OUT_DMA_ENGINE == "alt":
                        eng = nc.gpsimd if (rb * Y_SPLIT + yj) % 2 else nc.scalar
                    else:
                        eng = nc.gpsimd if OUT_DMA_ENGINE == "gpsimd" else nc.sync
                    ysl = slice(yj * YW, (yj + 1) * YW)
                    eng.dma_start(out=out_d[rsl, ysl], in_=yt[:, ysl])

        for _ in range(n_iter):
            one_iter()

    nc.compile()
    return nc


def _get_graph(k, n_iter=1):
    key = (k, n_iter, GROUP_OC, PSUM_BUFS, OUT_DMA_ENGINE, PROBE, Y_SPLIT,
           SCAN_SRC, W_DTYPE, XT_BUFS, A_BUFS, Y_BUFS)
    if key not in _graph_cache:
        _graph_cache[key] = _build(k, n_iter)
    return _graph_cache[key]


def _prep_weight(weight):
    w = np.asarray(weight, np.float32)
    wnp = ml_dtypes.float8_e4m3 if W_DTYPE == "fp8" else ml_dtypes.bfloat16
    key = (id(weight), w.shape, W_DTYPE,
           float(w[0, :16].sum()), float(w[-1, -16:].sum()), float(w.trace()))
    if key not in _w_cache:
        _w_cache.clear()
        wt = np.ascontiguousarray(w.T)
        _w_cache[key] = wt.astype(wnp)
    return _w_cache[key]


def _make_in_maps(input, weight):
    inp = np.asarray(input, np.float32)
    wt = _prep_weight(weight)
    inpT = np.ascontiguousarray(inp.T)            # [IN, BATCH]
    ah = inpT.astype(ml_dtypes.bfloat16)
    al = (inpT - ah.astype(np.float32)).astype(ml_dtypes.bfloat16)
    # pack [IN, BATCH] -> per-core [P, 2*KT*ROWS], col = split*KT*ROWS + kt*ROWS + r
    def pack(a, c):
        s = a[:, c * ROWS:(c + 1) * ROWS]                      # [IN, ROWS]
        return s.reshape(KT, P, ROWS).transpose(1, 0, 2).reshape(P, KT * ROWS)
    in_maps = []
    for c in range(N_CORES):
        in_maps.append({
            "act": np.ascontiguousarray(
                np.concatenate([pack(ah, c), pack(al, c)], axis=1)),
            "wt": wt,
        })
    return in_maps


def run_spmd(input, weight, hash_length, trace=False):
    """Run the SPMD kernel; returns (full_output, BassKernelResults)."""
    from concourse.bass_utils import run_bass_kernel_spmd
    k = int(hash_length)
    nc = _get_graph(k)
    in_maps = _make_in_maps(input, weight)
    res = run_bass_kernel_spmd(nc, in_maps, core_ids=list(range(N_CORES)), trace=trace)
    out = np.concatenate(
        [res.results[c]["out"].astype(np.float32) for c in range(N_CORES)], axis=0)
    return out, res


def kernel(input, weight, hash_length):
    out, _ = run_spmd(input, weight, hash_length, trace=False)
    return out


def make_bench_fn(input, weight, hash_length, n_iter):
    """Cached jitted shard_map over the n_iter-unrolled NEFF, with inputs
    uploaded once (not donated), for repeat-dispatch wall timing."""
    import jax
    import numpy as np_
    from jax.sharding import Mesh, PartitionSpec
    from jax.experimental.shard_map import shard_map
    from concourse import bass2jax
    import concourse.mybir as mybir

    bass2jax.install_neuronx_cc_hook()
    k = int(hash_length)
    nc = _get_graph(k, n_iter)
    in_maps = _make_in_maps(input, weight)

    part_name = nc.partition_id_tensor.name if nc.partition_id_tensor else None
    in_names, out_names, out_avals, zero_outs = [], [], [], []
    for alloc in nc.m.functions[0].allocations:
        if not isinstance(alloc, mybir.MemoryLocationSet):
            continue
        name = alloc.memorylocations[0].name
        if alloc.kind == "ExternalInput":
            if name != part_name:
                in_names.append(name)
        elif alloc.kind == "ExternalOutput":
            shape = tuple(alloc.tensor_shape)
            dtype = mybir.dt.np(alloc.dtype)
            out_names.append(name)
            out_avals.append(jax.core.ShapedArray(shape, dtype))
            zero_outs.append(np_.zeros((N_CORES * shape[0], *shape[1:]), dtype))
    n_params = len(in_names)
    all_names = in_names + out_names
    if part_name is not None:
        all_names = all_names + [part_name]

    def _body(*args):
        operands = list(args)
        if part_name is not None:
            operands.append(bass2jax.partition_id_tensor())
        outs = bass2jax._bass_exec_p.bind(
            *operands,
            out_avals=tuple(out_avals),
            in_names=tuple(all_names),
            out_names=tuple(out_names),
            lowering_input_output_aliases=(),
            sim_require_finite=True,
            sim_require_nnan=True,
            nc=nc,
        )
        return tuple(outs)

    devices = jax.devices()[:N_CORES]
    mesh = Mesh(np_.asarray(devices), ("core",))
    nin = n_params + len(out_names)
    fn = jax.jit(
        shard_map(_body, mesh=mesh,
                  in_specs=(PartitionSpec("core"),) * nin,
                  out_specs=(PartitionSpec("core"),) * len(out_names),
                  check_rep=False),
        keep_unused=True,
    )
    concat_in = [
        np_.concatenate([in_maps[c][nm] for c in range(N_CORES)], axis=0)
        for nm in in_names
    ]
    dev_args = [jax.device_put(a) for a in (*concat_in, *zero_outs)]
    jax.block_until_ready(dev_args)
    return fn, dev_args
